# revision 1
# baseline (speedup 1.0000x reference)
"""Trainium2 Bass kernel for nn_ModalDecoder (embedding_lookup).

Reference computation:
    w  = out_projection_table[idx].reshape(B, F, D, O)      # [B,F,D,O]
    b  = feature_bias_table[idx]                            # [B,F,D]
    xb = x[:, :, None, :] + b[:, None, :, :]                # [B,N,F,D]
    out = einsum('bnfd,bfdo->bnfo', xb, w)                  # [B,N,F,O]

Factorization (avoids the 128MB [B,N,F,D] intermediate):
    out[b, n, f, :] = x[b, n, :] @ W[b, f] + (bias[b, f] @ W[b, f])
The bias term is a per-(b,f) length-O vector, broadcast over n; it is
precomputed on host (B*F*D*O MACs, tiny) and added on-device per PSUM tile
via tensor_scalar_add (per-partition scalar).

Sharding: 8 cores = 4 values of b x 2 halves of N. Per core:
    y[fo, n] = Wpack[d, fo].T @ xT[d, n] + cvec[fo]
with Wpack = [D, F*O] (host-gathered tables packed side by side), xT the
transposed x half, both bf16 (PSUM accumulates fp32). y is [F*O, N/2] fp32.

Device kernel is raw Bass (no TileContext -> avoids Tile's expensive
kernel-tail drain + barrier butterfly). Manual semaphores. Loads go on the
sync-engine HWDGE ring in gating order (a lone DMA spreads across all 16
SDMA slots); stores alternate between the sync and scalar rings, with the
two tail groups split into 128KB halves across both rings. The PE is
pre-warmed with dummy matmuls during the load phase so the HAM clock gate
is at 2.4GHz when real matmuls start (warm issue rate: one 128x128x512
matmul per 216ns). No explicit end-of-kernel completion waits or semaphore
clears: the NEFF's own epilogue drains in-flight DMAs and re-zeroes every
semaphore, which also overlaps the last store transfer.

Per-core HBM traffic: 0.5MB xT + 1MB Wpack + 2MB out (memory-bound).
"""

import numpy as np
import ml_dtypes

B, N, D, O, F, V = 4, 1024, 512, 64, 16, 64
NH = N // 2            # 512 rows of x per core
FO = F * O             # 1024 packed output columns
KT = D // 128          # 4 contraction chunks
ST = FO // 128         # 8 output-partition chunks
N_WARM = 8             # PE warmup matmuls during load phase

_cache: dict = {}


def _build_program(with_clears=True):
    # with_clears=True is the real (HW) program. The False variant is for
    # CoreSim validation: it enables the race detector and memsets the
    # warmup scratch (CoreSim rejects reads of uninitialized SBUF; on HW
    # the warmup matmul inputs are garbage by design and never observed).
    import concourse.bass as bass
    import concourse.mybir as mybir

    bf16 = mybir.dt.bfloat16
    f32 = mybir.dt.float32

    nc = bass.Bass(
        "TRN2",
        target_bir_lowering=False,
        debug=False,
        num_devices=8,
        detect_race_conditions=not with_clears,
    )

    xt_d = nc.dram_tensor("xt", [128, KT * NH], bf16, kind="ExternalInput")
    wp_d = nc.dram_tensor("wp", [128, KT * FO], bf16, kind="ExternalInput")
    cv_d = nc.dram_tensor("cv", [128, ST], f32, kind="ExternalInput")
    # fp32 output: a bf16-output variant measured SLOWER (stores here are
    # descriptor/latency-bound, not byte-bound) and costs accuracy margin.
    y_d = nc.dram_tensor("y", [FO, NH], f32, kind="ExternalOutput")

    yv = y_d.ap().rearrange("(g p) n -> p g n", p=128)  # [128, ST, NH]

    with (
        nc.sbuf_tensor("xt_sb", [128, KT * NH], bf16) as xt_sb,
        nc.sbuf_tensor("wp_sb", [128, KT * FO], bf16) as wp_sb,
        nc.sbuf_tensor("cv_sb", [128, ST], f32) as cv_sb,
        nc.sbuf_tensor("out_sb", [128, ST, NH], f32) as out_sb,
        nc.sbuf_tensor("scr_sb", [128, NH], bf16) as scr_sb,
        nc.psum_tensor([128, ST, NH], f32) as ps,
        nc.semaphore("s_wp0") as s_wp0,
        nc.semaphore("s_wp1") as s_wp1,
        nc.semaphore("s_wp23") as s_wp23,
        nc.semaphore("s_wp45") as s_wp45,
        nc.semaphore("s_wp67") as s_wp67,
        nc.semaphore("s_xt01") as s_xt01,
        nc.semaphore("s_xt23") as s_xt23,
        nc.semaphore("s_cv") as s_cv,
        nc.semaphore("s_ws") as s_ws,
        nc.semaphore("s_mm") as s_mm,
        nc.semaphore("s_dve_sync") as s_dve_sync,
        nc.semaphore("s_dve_act") as s_dve_act,
        nc.semaphore("s_st_sync") as s_st_sync,
        nc.semaphore("s_st_act") as s_st_act,
        nc.Block() as block,
    ):

        @block.sync
        def _(sync):
            # All loads on one ring in gating order: a lone DMA spreads over
            # all 16 SDMA slots, so serial-on-one-ring beats split-across-two
            # for time-to-first-gate. Ring FIFO keeps completion in order;
            # xt and wp chunks are interleaved so PE can open group 0 after
            # only 512KB has landed and every later gate arrives just in time
            # (PE consumes one 128KB wp group per ~860ns).
            sync.dma_start(xt_sb[:, 0:1024], xt_d.ap()[:, 0:1024]).then_inc(
                s_xt01, 16
            )
            sync.dma_start(wp_sb[:, 0:512], wp_d.ap()[:, 0:512]).then_inc(s_wp0, 16)
            sync.dma_start(xt_sb[:, 1024:2048], xt_d.ap()[:, 1024:2048]).then_inc(
                s_xt23, 16
            )
            sync.dma_start(wp_sb[:, 512:1024], wp_d.ap()[:, 512:1024]).then_inc(
                s_wp1, 16
            )
            sync.dma_start(wp_sb[:, 1024:2048], wp_d.ap()[:, 1024:2048]).then_inc(
                s_wp23, 16
            )
            sync.dma_start(wp_sb[:, 2048:3072], wp_d.ap()[:, 2048:3072]).then_inc(
                s_wp45, 16
            )
            sync.dma_start(wp_sb[:, 3072:4096], wp_d.ap()[:, 3072:4096]).then_inc(
                s_wp67, 16
            )
            for j, s in enumerate((0, 2, 4)):
                sync.wait_ge(s_dve_sync, j + 1)
                sync.dma_start(yv[:, s, :], out_sb[:, s, :]).then_inc(s_st_sync, 16)
            # Tail groups 6/7 are stored in 128KB halves split across both
            # rings so the final transfer after the last matmul is short.
            sync.wait_ge(s_dve_sync, 4)
            sync.dma_start(yv[:, 6, 0:256], out_sb[:, 6, 0:256]).then_inc(
                s_st_sync, 16
            )
            sync.wait_ge(s_dve_sync, 5)
            sync.dma_start(yv[:, 7, 0:256], out_sb[:, 7, 0:256]).then_inc(
                s_st_sync, 16
            )
            # No final completion wait: the framework epilogue's DRAIN retires
            # in-flight DMAs, and its semaphore sweep re-zeroes every sem.

        @block.scalar
        def _(scalar):
            # cv also primes this ring's DMA path before the stores.
            scalar.dma_start(cv_sb[:], cv_d.ap()).then_inc(s_cv, 16)
            for j, s in enumerate((1, 3, 5)):
                scalar.wait_ge(s_dve_act, j + 1)
                scalar.dma_start(yv[:, s, :], out_sb[:, s, :]).then_inc(s_st_act, 16)
            scalar.wait_ge(s_dve_act, 4)
            scalar.dma_start(yv[:, 6, 256:512], out_sb[:, 6, 256:512]).then_inc(
                s_st_act, 16
            )
            scalar.wait_ge(s_dve_act, 5)
            scalar.dma_start(yv[:, 7, 256:512], out_sb[:, 7, 256:512]).then_inc(
                s_st_act, 16
            )

        @block.tensor
        def _(tensor):
            # Warm the PE HAM clock gate while loads are in flight. scr_sb is
            # never written on HW (garbage is fine — the warmup PSUM bank is
            # overwritten with start=True by group ST-1 before any read); the
            # sim variant memsets it because CoreSim rejects uninit reads.
            if not with_clears:
                tensor.wait_ge(s_ws, 1)
            for _ in range(N_WARM):
                nc.tensor.matmul(
                    ps[:, ST - 1, :],
                    scr_sb[:, :128],
                    scr_sb[:],
                    start=True,
                    stop=True,
                )
            # Group-serial accumulation: group s finishes after its own 4
            # matmuls, so DVE adds + stores pipeline behind PE. wp columns
            # are laid out [s][k][fo_local].
            tensor.wait_ge(s_xt01, 16)
            for s in range(ST):
                if s == 0:
                    tensor.wait_ge(s_wp0, 16)
                elif s == 1:
                    tensor.wait_ge(s_wp1, 16)
                elif s == 2:
                    tensor.wait_ge(s_wp23, 16)
                elif s == 4:
                    tensor.wait_ge(s_wp45, 16)
                elif s == 6:
                    tensor.wait_ge(s_wp67, 16)
                for k in range(KT):
                    if s == 0 and k == 2:
                        tensor.wait_ge(s_xt23, 16)
                    inst = nc.tensor.matmul(
                        ps[:, s, :],
                        wp_sb[:, s * 512 + k * 128:s * 512 + (k + 1) * 128],
                        xt_sb[:, k * NH:(k + 1) * NH],
                        start=(k == 0),
                        stop=(k == KT - 1),
                    )
                    if k == KT - 1:
                        inst.then_inc(s_mm, 1)

        @block.vector
        def _(vector):
            if not with_clears:
                vector.memset(scr_sb[:], 0).then_inc(s_ws, 1)
            vector.wait_ge(s_cv, 16)  # cv loaded
            for s in range(ST - 2):
                vector.wait_ge(s_mm, s + 1)
                inst = nc.vector.tensor_scalar_add(
                    out_sb[:, s, :], ps[:, s, :], cv_sb[:, s:s + 1]
                )
                if s % 2 == 0:
                    inst.then_inc(s_dve_sync, 1)
                else:
                    inst.then_inc(s_dve_act, 1)
            for s in (ST - 2, ST - 1):
                vector.wait_ge(s_mm, s + 1)
                for h, sem in ((0, s_dve_sync), (1, s_dve_act)):
                    nc.vector.tensor_scalar_add(
                        out_sb[:, s, h * 256:(h + 1) * 256],
                        ps[:, s, h * 256:(h + 1) * 256],
                        cv_sb[:, s:s + 1],
                    ).then_inc(sem, 1)

    return nc


def _get_program():
    nc = _cache.get("nc")
    if nc is None:
        nc = _build_program()
        _cache["nc"] = nc
    return nc


def _prep_in_maps(x, idx, fbt, opt):
    bf = ml_dtypes.bfloat16
    in_maps = []
    for b in range(B):
        w = opt[idx[b]].reshape(F, D, O)                     # [F,D,O] f32
        wpack = w.transpose(1, 0, 2).reshape(KT, 128, ST, 128)  # [k,p,s,c]
        wp_host = np.ascontiguousarray(
            wpack.transpose(1, 2, 0, 3).reshape(128, KT * FO)
        ).astype(bf)                                         # [p, s*512+k*128+c]
        bias = fbt[idx[b]]                                   # [F,D]
        cvec = np.einsum("fd,fdo->fo", bias, w).reshape(FO).astype(np.float32)
        cv = np.ascontiguousarray(cvec.reshape(ST, 128).T)   # [128, ST]
        for h in range(2):
            xtT = x[b, h * NH:(h + 1) * NH, :].T             # [D, NH]
            xt_host = np.ascontiguousarray(
                xtT.reshape(KT, 128, NH).transpose(1, 0, 2).reshape(128, KT * NH)
            ).astype(bf)                                     # [128, KT*NH]
            in_maps.append({"xt": xt_host, "wp": wp_host, "cv": cv})
    return in_maps


def _assemble(results):
    out = np.empty((B, N, F, O), dtype=np.float32)
    for c in range(8):
        b, h = divmod(c, 2)
        y = np.asarray(results[c]["y"])                      # [FO, NH]
        out[b, h * NH:(h + 1) * NH] = y.reshape(F, O, NH).transpose(2, 0, 1)
    return out


def _run(x, idx, feature_bias_table, out_projection_table, **run_kwargs):
    from concourse.bass_utils import run_bass_kernel_spmd

    x = np.asarray(x, dtype=np.float32)
    idx = np.asarray(idx).astype(np.int64)
    fbt = np.asarray(feature_bias_table, dtype=np.float32)
    opt = np.asarray(out_projection_table, dtype=np.float32)

    nc = _get_program()
    in_maps = _prep_in_maps(x, idx, fbt, opt)
    res = run_bass_kernel_spmd(nc, in_maps, core_ids=list(range(8)), **run_kwargs)
    return _assemble(res.results), res


def kernel(x, idx, feature_bias_table, out_projection_table):
    out, _ = _run(x, idx, feature_bias_table, out_projection_table)
    return out

